# revision 23
# baseline (speedup 1.0000x reference)
"""Trainium2 Bass kernel v3 for DeepGCN (nn_DeepGCN_82454782148693).

8-core SPMD, dst-sharded, fp16 data path with binary-fp8 scatter matrices.
Per layer:
  P1 (software-pipelined with P5 of the previous layer): hT = conv(xT)
      (fp16 matmul, f32 psum); PE-transpose per tile; scale by dinv_src;
      write node-major fp16 shard rows into 4 per-quarter hsh tensors.
      Quarter AllGathers fire as soon as each quarter's rows are written,
      overlapping the rest of P1 and the gather phase.
  P3: slots = edges+self-loops sorted by (bank, tile); bank = src-node local
      quarter (keeps per-core self-loop counts symmetric, ~6% padding).
      dma_gather calls (2048 idx, queue_num=bank spreads SWDGE descgen over
      all four Q7 core pairs) pull message rows from the AllGathered halls;
      host-built BINARY fp8 S streams from HBM; PE: agg[f, dst-tile] +=
      matmul(lhsT=msgs, rhs=S). dinv_dst applied during the psum->sbuf copy
      (dinvb broadcast tile). Lin matmul per 2-tile window; h2 fp16 resident;
      BN stats via accum_out. conv_b/lin_b dropped (absorbed exactly by BN).
  P4: tiny stats AllReduce; fold BN affine + C1.
  P5: xT = relu(C1*bn(h2) + alpha*x0 + theta*xT); x0 streamed from HBM.
Classifier feature-major, f32 out [2, 12544]/core.
"""

import os
import sys

import numpy as np

for _p in ("/opt/trn_rl_repo", "/root/.axon_site/_ro/trn_rl_repo"):
    if os.path.isdir(_p) and _p not in sys.path:
        sys.path.append(_p)

import concourse.bass as bass
import concourse.bacc as bacc
import concourse.mybir as mybir
import concourse.tile as tile
from concourse import bass_utils

F32 = mybir.dt.float32
F16 = mybir.dt.float16
F8 = mybir.dt.float8e4
I16 = mybir.dt.int16
AF = mybir.ActivationFunctionType
OP = mybir.AluOpType
AX = mybir.AxisListType

N = 100000
NCORES = 8
NSH = N // NCORES
TILES = 98
NPAD = TILES * 128
NP = NCORES * NPAD
BANKS = 4
BLOC = [2048, 3584, 3584, 3328]     # local rows per bank (512-aligned)
BOFF = [0, 2048, 5632, 9216]
H = 128
L, HC, C = 4, 64, 2
ALPHA, THETA, EPS = 0.1, 0.5, 1e-5
C1 = float(1.0 - ALPHA - THETA)
WTILES = 2
GSUB = 16         # chunks per dma_gather call


# ----------------------------------------------------------------------------
# Host preprocessing (SPMD-common schedule)
# ----------------------------------------------------------------------------

def host_prep(edge_index):
    src0 = np.asarray(edge_index[0], np.int64)
    dst0 = np.asarray(edge_index[1], np.int64)
    loops = np.arange(N, dtype=np.int64)
    src = np.concatenate([src0, loops])
    dst = np.concatenate([dst0, loops])

    deg = np.bincount(dst0, minlength=N).astype(np.float32) + 1.0
    dinv = (1.0 / np.sqrt(deg)).astype(np.float32)

    core = dst // NSH
    tile_id = (dst - core * NSH) >> 7
    part = (dst - core * NSH) & 127
    j = src % NSH
    boffs = np.array(BOFF + [NSH], np.int64)
    bank = np.searchsorted(boffs, j, side="right") - 1
    bank = np.minimum(bank, 3)
    blocs = np.array(BLOC, np.int64)
    bidx = ((src // NSH) * blocs[bank] + (j - boffs[bank])).astype(np.int64)

    # counts per (core, bank, tile)
    cnt = np.zeros((NCORES, BANKS, TILES), np.int64)
    np.add.at(cnt, (core, bank, tile_id), 1)
    nmax = cnt.max(axis=0)                      # [BANKS, TILES]

    # common slot layout: bank stream = concat_t nmax[b, t]; bank end padded
    # to a multiple of 128.
    bucket0 = np.zeros((BANKS, TILES + 1), np.int64)   # slot offset in bank
    bank_slots = np.zeros(BANKS, np.int64)
    for b in range(BANKS):
        bucket0[b, 1:] = np.cumsum(nmax[b])
        bank_slots[b] = ((bucket0[b, -1] + 127) // 128) * 128
    bank_c = bank_slots // 128                  # chunks per bank
    bank_chunk0 = np.concatenate([[0], np.cumsum(bank_c)])
    Ctot = int(bank_c.sum())
    TOT = Ctot * 128
    bank_slot0 = bank_chunk0 * 128              # global slot offset per bank

    # common (chunk, tile) schedule, chunk-major si enumeration
    sched = []        # (global_chunk, tile, si, bank)
    si = 0
    tile_nmm = np.zeros(TILES, np.int64)
    for b in range(BANKS):
        for ch in range(int(bank_c[b])):
            s0, s1 = ch * 128, (ch + 1) * 128
            t0 = int(np.searchsorted(bucket0[b], s0, side="right") - 1)
            t1 = int(np.searchsorted(bucket0[b], s1 - 1, side="right") - 1)
            t1 = min(t1, TILES - 1)
            for t in range(t0, t1 + 1):
                if bucket0[b, t + 1] > s0 and bucket0[b, t] < s1 \
                        and nmax[b, t] > 0:
                    sched.append((int(bank_chunk0[b] + ch), t, si, b))
                    tile_nmm[t] += 1
                    si += 1
    NS = si

    # per-core slot fill
    order = np.argsort(((core * BANKS + bank) * TILES + tile_id), kind="stable")
    co, bo, to_, po, io = (core[order], bank[order], tile_id[order],
                           part[order], bidx[order])
    # position within (core, bank, tile)
    keyo = ((co * BANKS + bo) * TILES + to_)
    runstart = np.r_[0, np.flatnonzero(np.diff(keyo)) + 1]
    runid = np.zeros(len(keyo), np.int64)
    runid[runstart[1:]] = 1
    runid = np.cumsum(runid)
    pos = np.arange(len(keyo)) - runstart[runid]
    slot_global = bank_slot0[bo] + bucket0[bo, to_] + pos

    si_map = {}
    for (chg, t, s, b) in sched:
        si_map[(chg, t)] = s

    cores = []
    for c in range(NCORES):
        sel = co == c
        sl = slot_global[sel]
        idx = np.zeros(TOT, np.int16)
        idx[sl] = io[sel].astype(np.int16)
        Sarr = np.zeros((128, NS * 128), np.uint8)
        ch_of = sl >> 7
        si_of = np.array([si_map[(int(chv), int(tv))]
                          for chv, tv in zip(ch_of, to_[sel])], np.int64)
        Sarr[sl & 127, (si_of << 7) + po[sel]] = 0x38  # fp8e4m3 1.0

        idx16 = idx.reshape(TOT // 16, 16).T
        idx16 = np.tile(idx16, (8, 1)).astype(np.int16)
        cores.append(dict(idx16=np.ascontiguousarray(idx16),
                          S=np.ascontiguousarray(Sarr)))

    meta = dict(Ctot=Ctot, NS=NS, sched=sched, tile_nmm=tile_nmm,
                bank_c=bank_c.astype(np.int64),
                bank_chunk0=bank_chunk0.astype(np.int64))
    return dinv, meta, cores


def pack_weights(inputs):
    cols = [np.asarray(inputs["proj_W"], np.float32)]
    for l in range(L):
        cols.append(np.asarray(inputs["conv_W"][l], np.float32))
    for l in range(L):
        cols.append(np.asarray(inputs["lin_W"][l], np.float32))
    cols.append(np.asarray(inputs["cls_W1"], np.float32))
    w2 = np.zeros((H, C), np.float32)
    w2[:HC] = np.asarray(inputs["cls_W2"], np.float32)
    cols.append(w2)
    W = np.concatenate(cols, axis=1).astype(np.float16)
    nb = np.zeros((H, 7), np.float32)
    nb[:, 0] = np.asarray(inputs["proj_b"], np.float32)
    nb[:HC, 5] = np.asarray(inputs["cls_b1"], np.float32)
    nb[:C, 6] = np.asarray(inputs["cls_b2"], np.float32)
    bn = np.zeros((H, 2 * L), np.float32)
    for l in range(L):
        bn[:, l] = np.asarray(inputs["bn_g"][l], np.float32)
        bn[:, L + l] = np.asarray(inputs["bn_b"][l], np.float32)
    return W, nb, bn


# ----------------------------------------------------------------------------
# Device program
# ----------------------------------------------------------------------------

def build_program(meta):
    Ctot, NS = meta["Ctot"], meta["NS"]
    sched = meta["sched"]
    tile_nmm = meta["tile_nmm"]
    bank_c = meta["bank_c"]
    bank_chunk0 = meta["bank_chunk0"]
    WCOLS = H * (1 + 2 * L) + HC + C
    IDXCOLS = Ctot * 128 // 16

    nc = bacc.Bacc("TRN2", target_bir_lowering=False, debug=False,
                   enable_asserts=False, num_devices=NCORES,
                   num_swdge_queues=4)

    xT_in = nc.dram_tensor("xT_in", [H, NPAD], F16, kind="ExternalInput").ap()
    dinv_in = nc.dram_tensor("dinv_in", [H, TILES], F32,
                             kind="ExternalInput").ap()
    idx_in = nc.dram_tensor("idx_in", [H, IDXCOLS], I16,
                            kind="ExternalInput").ap()
    s_in = nc.dram_tensor("s_in", [H, NS * H], F8, kind="ExternalInput").ap()
    dinvb_in = nc.dram_tensor("dinvb_in", [H, NPAD], F16,
                              kind="ExternalInput").ap()
    w_in = nc.dram_tensor("w_in", [H, WCOLS], F16, kind="ExternalInput").ap()
    b_in = nc.dram_tensor("b_in", [H, 7], F32, kind="ExternalInput").ap()
    bn_in = nc.dram_tensor("bn_in", [H, 2 * L], F32, kind="ExternalInput").ap()
    out_d = nc.dram_tensor("out_d", [C, NPAD], F32, kind="ExternalOutput").ap()

    hsh_ds = [nc.dram_tensor(f"hsh{b}_d", [BLOC[b], H], F16,
                             kind="Internal").ap() for b in range(BANKS)]
    x0_d = nc.dram_tensor("x0_d", [H, NPAD], F16, kind="Internal").ap()
    hall_ds = [nc.dram_tensor(f"hall{b}_d", [NCORES * BLOC[b], H], F16,
                              kind="Internal", addr_space="Shared").ap()
               for b in range(BANKS)]
    stin_d = nc.dram_tensor("stin_d", [H, 2], F32, kind="Internal").ap()
    stout_d = nc.dram_tensor("stout_d", [H, 2], F32, kind="Internal",
                             addr_space="Shared").ap()

    xT = nc.alloc_sbuf_tensor("xT", [H, NPAD], F16).ap()
    dinvb = nc.alloc_sbuf_tensor("dinvb", [H, NPAD], F16).ap()
    h2 = nc.alloc_sbuf_tensor("h2", [H, NPAD], F16).ap()
    idxs = nc.alloc_sbuf_tensor("idxs", [H, IDXCOLS], I16).ap()
    wsb = nc.alloc_sbuf_tensor("wsb", [H, WCOLS], F16).ap()
    bsb = nc.alloc_sbuf_tensor("bsb", [H, 7], F32).ap()
    bnsb = nc.alloc_sbuf_tensor("bnsb", [H, 2 * L], F32).ap()
    dinv = nc.alloc_sbuf_tensor("dinv", [H, TILES], F32).ap()
    ident = nc.alloc_sbuf_tensor("ident", [H, H], F16).ap()
    sums = nc.alloc_sbuf_tensor("sums", [H, 64], F32).ap()
    sqs = nc.alloc_sbuf_tensor("sqs", [H, 64], F32).ap()
    stat = nc.alloc_sbuf_tensor("stat", [H, 12], F32).ap()

    wproj = wsb[:, 0:H]
    wconv = lambda l: wsb[:, H * (1 + l):H * (2 + l)]
    wlin = lambda l: wsb[:, H * (1 + L + l):H * (2 + L + l)]
    wcls1 = wsb[:, H * (1 + 2 * L):H * (1 + 2 * L) + HC]
    wcls2 = wsb[:HC, H * (1 + 2 * L) + HC:WCOLS]

    rg = [list(range(NCORES))]
    hall_banks = hall_ds
    PCH = [(o, min(512, NPAD - o)) for o in range(0, NPAD, 512)]
    AGCI = {3: 0, 10: 1, 17: 2, 24: 3}

    # window structure: WTILES tiles each; per (window, bank): matmul list
    windows = []
    for t0 in range(0, TILES, WTILES):
        t1 = min(t0 + WTILES, TILES)
        per_bank = []
        for b in range(BANKS):
            mms = [e for e in sched if e[3] == b and t0 <= e[1] < t1]
            per_bank.append(mms)
        windows.append((t0, t1, per_bank))

    with tile.TileContext(nc) as tc:
        # ================= P0 =================
        with tc.sbuf_pool(name="p0", bufs=3) as pool, \
             tc.psum_pool(name="p0p", bufs=2) as pp:
            nc.sync.dma_start(wsb, w_in)
            nc.sync.dma_start(bsb, b_in)
            nc.sync.dma_start(bnsb, bn_in)
            nc.sync.dma_start(dinv, dinv_in)
            nc.sync.dma_start(idxs, idx_in)
            nc.sync.dma_start(dinvb, dinvb_in)
            iota = pool.tile([H, H], F32, tag="iota")
            pidx = pool.tile([H, H], F32, tag="pidx")
            nc.gpsimd.iota(iota, pattern=[[1, H]], base=0, channel_multiplier=0,
                           allow_small_or_imprecise_dtypes=True)
            nc.gpsimd.iota(pidx, pattern=[[0, H]], base=0, channel_multiplier=1,
                           allow_small_or_imprecise_dtypes=True)
            nc.vector.tensor_tensor(ident, iota, pidx, OP.is_equal)

        def emit_p0_chunk(pool, pp, off, w):
            xin = pool.tile([H, 512], F16, tag="xin")
            nc.sync.dma_start(xin[:, :w], xT_in[:, off:off + w])
            ps = pp.tile([H, 512], F32, tag="ps0")
            nc.tensor.matmul(ps[:, :w], wproj, xin[:, :w])
            nc.scalar.activation(xT[:, off:off + w], ps[:, :w], AF.Relu,
                                 bias=bsb[:, 0:1], scale=1.0)
            x0t = pool.tile([H, 512], F16, tag="x0t")
            nc.scalar.mul(x0t[:, :w], xT[:, off:off + w], ALPHA)
            if off + w > NSH:
                nc.vector.memset(x0t[:, max(0, NSH - off):w], 0.0)
                nc.vector.memset(xT[:, NSH:off + w], 0.0)
            nc.sync.dma_start(x0_d[:, off:off + w], x0t[:, :w])

        def emit_p5_chunk(pool, off, w):
            x0t = pool.tile([H, 512], F16, tag="x0t5")
            nc.sync.dma_start(x0t[:, :w], x0_d[:, off:off + w])
            t1_ = pool.tile([H, 512], F32, tag="t1")
            nc.vector.tensor_scalar(t1_[:, :w], h2[:, off:off + w],
                                    stat[:, 6:7], stat[:, 7:8],
                                    op0=OP.mult, op1=OP.add)
            t2 = pool.tile([H, 512], F32, tag="t2")
            nc.vector.scalar_tensor_tensor(
                t2[:, :w], xT[:, off:off + w], THETA, t1_[:, :w],
                op0=OP.mult, op1=OP.add)
            t3 = pool.tile([H, 512], F32, tag="t3")
            nc.vector.tensor_tensor(t3[:, :w], t2[:, :w], x0t[:, :w], OP.add)
            nc.scalar.activation(xT[:, off:off + w], t3[:, :w], AF.Relu,
                                 bias=0.0, scale=1.0)
            if off + w > NSH:
                nc.vector.memset(xT[:, NSH:off + w], 0.0)

        for li in range(L):
            # ---- P1 (software-pipelined with P5 of previous layer) ----
            with tc.sbuf_pool(name=f"l{li}a", bufs=3) as pool, \
                 tc.psum_pool(name=f"l{li}ap", bufs=2) as pp, \
                 tc.psum_pool(name=f"l{li}at", bufs=4) as pt:
                if li > 0:
                    emit_p5_chunk(pool, *PCH[0])
                else:
                    emit_p0_chunk(pool, pp, *PCH[0])
                for ci, (off, w) in enumerate(PCH):
                    if ci + 1 < len(PCH):
                        if li > 0:
                            emit_p5_chunk(pool, *PCH[ci + 1])
                        else:
                            emit_p0_chunk(pool, pp, *PCH[ci + 1])
                    ps = pp.tile([H, 512], F32, tag="ps")
                    nc.tensor.matmul(ps[:, :w], wconv(li), xT[:, off:off + w])
                    hT = pool.tile([H, 512], F16, tag="hT")
                    nc.vector.tensor_copy(hT[:, :w], ps[:, :w])
                    stg = pool.tile([H, 512], F16, tag="stg")
                    for j in range(w // 128):
                        t = off // 128 + j
                        tp2 = pt.tile([H, H], F16, tag="tp2")
                        nc.tensor.transpose(tp2, hT[:, j * 128:(j + 1) * 128],
                                            ident)
                        nc.scalar.mul(stg[:, j * 128:(j + 1) * 128], tp2,
                                      dinv[:, t:t + 1])
                    # write rows [off, off+w) split by bank boundary
                    r = off
                    while r < off + w:
                        b = max(i for i in range(BANKS) if BOFF[i] <= r)
                        rend = min(off + w, BOFF[b] + BLOC[b])
                        dram = hsh_ds[b][r - BOFF[b]:rend - BOFF[b], :]\
                            .rearrange("(j p) f -> p j f", p=128)
                        c0, c1 = r - off, rend - off
                        nc.sync.dma_start(
                            dram, stg[:, c0:c1].rearrange(
                                "p (j f) -> p j f", f=H))
                        r = rend
                    if ci in AGCI:
                        k = AGCI[ci]
                        nc.gpsimd.collective_compute(
                            "AllGather", OP.bypass, replica_groups=rg,
                            ins=[hsh_ds[k]], outs=[hall_ds[k]])

            # ---- P3 ----
            with tc.sbuf_pool(name=f"l{li}g", bufs=3) as gpool, \
                 tc.sbuf_pool(name=f"l{li}s", bufs=3) as spool, \
                 tc.sbuf_pool(name=f"l{li}h", bufs=3) as hpool, \
                 tc.psum_pool(name=f"l{li}pa", bufs=5) as ppa, \
                 tc.psum_pool(name=f"l{li}pl", bufs=2) as ppl:
                msgs_of = {}       # global chunk -> (tile_handle, local_off)
                next_call = [int(bank_chunk0[b]) for b in range(BANKS)]

                def issue_call(b):
                    c0 = next_call[b]
                    cn = min(GSUB, int(bank_chunk0[b + 1]) - c0)
                    if cn <= 0:
                        return
                    mt = gpool.tile([H, GSUB * H], F16, tag=f"m{b}")
                    nc.gpsimd.dma_gather(
                        out_ap=mt[:, :cn * H].rearrange("p (c f) -> p c f",
                                                        f=H),
                        in_ap=hall_banks[b],
                        idxs_ap=idxs[:, c0 * 8:(c0 + cn) * 8],
                        num_idxs=cn * 128, num_idxs_reg=cn * 128,
                        elem_size=H, single_packet=False, queue_num=b)
                    for k in range(cn):
                        msgs_of[c0 + k] = (mt, k)
                    next_call[b] = c0 + cn

                def ensure_gathered(chg, b):
                    while chg >= next_call[b]:
                        issue_call(b)

                for _ in range(3):
                    for b in (0, 1, 2):
                        issue_call(b)

                tile_done = np.zeros(TILES, np.int64)
                psum_of = {}
                sc = 0
                for (t0, t1, per_bank) in windows:
                    for b in range(BANKS):
                        mms = per_bank[b]
                        if not mms:
                            continue
                        si0, si1 = mms[0][2], mms[-1][2] + 1
                        st = spool.tile([H, 16 * H], F8, tag=f"s{b}")
                        ns = si1 - si0
                        assert ns <= 16, (t0, b, ns)
                        nc.sync.dma_start(st[:, :ns * H],
                                          s_in[:, si0 * H:si1 * H])
                        for (chg, t, si, _b) in mms:
                            ensure_gathered(chg, b)
                            mt, lo = msgs_of[chg]
                            if t not in psum_of:
                                psum_of[t] = ppa.tile([H, H], F32, tag="agg",
                                                      name="agg")
                            first = tile_done[t] == 0
                            last = tile_done[t] == tile_nmm[t] - 1
                            nc.tensor.matmul(
                                psum_of[t],
                                mt[:, lo * H:(lo + 1) * H],
                                st[:, (si - si0) * H:(si - si0 + 1) * H],
                                start=bool(first), stop=bool(last),
                                skip_group_check=True)
                            tile_done[t] += 1
                    # finalize closed tiles of this window
                    nwc = (t1 - t0) * H
                    aggs = hpool.tile([H, WTILES * H], F16, tag="aggs")
                    for t in range(t0, t1):
                        if t in psum_of:
                            nc.vector.tensor_tensor(
                                aggs[:, (t - t0) * H:(t - t0 + 1) * H],
                                psum_of.pop(t),
                                dinvb[:, t * H:(t + 1) * H], OP.mult)
                        else:
                            nc.vector.memset(
                                aggs[:, (t - t0) * H:(t - t0 + 1) * H], 0.0)
                    ps3 = ppl.tile([H, WTILES * H], F32, tag="ps3")
                    nc.tensor.matmul(ps3[:, :nwc], wlin(li), aggs[:, :nwc])
                    gcol = t0 * H
                    nc.vector.tensor_scalar(
                        h2[:, gcol:gcol + nwc], ps3[:, :nwc], 0.0, None,
                        op0=OP.add, op1=OP.add,
                        accum_out=sums[:, sc:sc + 1])
                    sq = hpool.tile([H, WTILES * H], F16, tag="sq")
                    nc.vector.scalar_tensor_tensor(
                        sq[:, :nwc], h2[:, gcol:gcol + nwc], 0.0,
                        h2[:, gcol:gcol + nwc],
                        op0=OP.add, op1=OP.mult, accum_out=sqs[:, sc:sc + 1])
                    sc += 1
                    if sc > 64:
                        raise RuntimeError("stats overflow")
                nparts = sc

            # ---- P4 ----
            with tc.sbuf_pool(name=f"l{li}r", bufs=2) as pool:
                nc.vector.tensor_reduce(stat[:, 0:1], sums[:, :nparts], AX.X,
                                        OP.add)
                nc.vector.tensor_reduce(stat[:, 1:2], sqs[:, :nparts], AX.X,
                                        OP.add)
                nc.sync.dma_start(stin_d, stat[:, 0:2])
                nc.gpsimd.collective_compute(
                    "AllReduce", OP.add, replica_groups=rg,
                    ins=[stin_d], outs=[stout_d])
                nc.sync.dma_start(stat[:, 2:4], stout_d)
                invn = 1.0 / float(N)
                nc.vector.tensor_scalar_mul(stat[:, 4:5], stat[:, 2:3], invn)
                m2 = pool.tile([H, 1], F32)
                nc.vector.tensor_tensor(m2, stat[:, 4:5], stat[:, 4:5],
                                        OP.mult)
                nc.vector.scalar_tensor_tensor(stat[:, 5:6], stat[:, 3:4],
                                               invn, m2, op0=OP.mult,
                                               op1=OP.subtract)
                vps = pool.tile([H, 1], F32)
                nc.vector.tensor_scalar_add(vps, stat[:, 5:6], float(EPS))
                sd = pool.tile([H, 1], F32)
                nc.scalar.sqrt(sd, vps)
                inv = pool.tile([H, 1], F32)
                nc.vector.reciprocal(inv, sd)
                gi = pool.tile([H, 1], F32)
                nc.vector.tensor_tensor(gi, inv, bnsb[:, li:li + 1], OP.mult)
                nc.vector.tensor_scalar_mul(stat[:, 6:7], gi, C1)
                ms = pool.tile([H, 1], F32)
                nc.vector.tensor_tensor(ms, stat[:, 4:5], stat[:, 6:7],
                                        OP.mult)
                nc.vector.scalar_tensor_tensor(
                    stat[:, 7:8], bnsb[:, L + li:L + li + 1], C1, ms,
                    op0=OP.mult, op1=OP.subtract)

        # ================= P6 (fused with final P5) =================
        with tc.sbuf_pool(name="p6", bufs=3) as pool, \
             tc.psum_pool(name="p6p", bufs=2) as pp, \
             tc.psum_pool(name="p6q", bufs=2) as pq:
            emit_p5_chunk(pool, *PCH[0])
            for ci, (off, w) in enumerate(PCH):
                if ci + 1 < len(PCH):
                    emit_p5_chunk(pool, *PCH[ci + 1])
                ps = pp.tile([HC, 512], F32, tag="ps")
                nc.tensor.matmul(ps[:, :w], wcls1, xT[:, off:off + w])
                h3 = pool.tile([HC, 512], F16, tag="h3")
                nc.scalar.activation(h3[:, :w], ps[:, :w], AF.Relu,
                                     bias=bsb[:HC, 5:6], scale=1.0)
                ps2 = pq.tile([C, 512], F32, tag="ps2")
                nc.tensor.matmul(ps2[:, :w], wcls2, h3[:, :w])
                ot = pool.tile([C, 512], F32, tag="ot")
                nc.vector.tensor_scalar(ot[:, :w], ps2[:, :w],
                                        bsb[:C, 6:7], None, op0=OP.add)
                nc.sync.dma_start(out_d[:, off:off + w], ot[:, :w])

    nc.compile()
    return nc


# ----------------------------------------------------------------------------
# Orchestration
# ----------------------------------------------------------------------------

LAST_RESULTS = None
_PROGRAM_CACHE = {}


def kernel(**inputs):
    global LAST_RESULTS
    edge_index = np.asarray(inputs["edge_index"])
    dinv, meta, cores = host_prep(edge_index)
    W, NB, BN = pack_weights(inputs)

    x = np.asarray(inputs["x"], np.float32)
    dl = np.ones(NCORES * NPAD, np.float32)
    for c in range(NCORES):
        dl[c * NPAD:c * NPAD + NSH] = dinv[c * NSH:(c + 1) * NSH]
    dinv_nm = dl.reshape(NCORES, TILES, 128).transpose(0, 2, 1)
    dz = np.zeros(NCORES * NPAD, np.float16)
    for c in range(NCORES):
        dz[c * NPAD:c * NPAD + NSH] = dinv[c * NSH:(c + 1) * NSH]
    dinvb_nm = dz.reshape(NCORES, NPAD)

    in_maps = []
    for c in range(NCORES):
        xs = np.zeros((H, NPAD), np.float16)
        xs[:, :NSH] = x[c * NSH:(c + 1) * NSH].T.astype(np.float16)
        in_maps.append({
            "xT_in": xs,
            "dinv_in": np.ascontiguousarray(dinv_nm[c]),
            "idx_in": cores[c]["idx16"],
            "s_in": cores[c]["S"],
            "dinvb_in": np.ascontiguousarray(
                np.broadcast_to(dinvb_nm[c], (H, NPAD))),
            "w_in": W,
            "b_in": NB,
            "bn_in": BN,
        })

    key = (meta["Ctot"], meta["NS"])
    if key not in _PROGRAM_CACHE:
        _PROGRAM_CACHE[key] = build_program(meta)
    nc = _PROGRAM_CACHE[key]

    trace = bool(int(os.environ.get("GCN_TRACE", "0")))
    res = bass_utils.run_bass_kernel_spmd(
        nc, in_maps, core_ids=list(range(NCORES)), trace=trace)
    LAST_RESULTS = res

    out = np.empty((N, C), np.float32)
    for c in range(NCORES):
        o = res.results[c]["out_d"]
        out[c * NSH:(c + 1) * NSH] = o[:, :NSH].T
    return out


# revision 25
# speedup vs baseline: 1.0324x; 1.0324x over previous
"""Trainium2 Bass kernel v3 for DeepGCN (nn_DeepGCN_82454782148693).

8-core SPMD, dst-sharded, fp16 data path with binary-fp8 scatter matrices.
9.32ms (baseline) -> 3.17ms on trn2.8x1. Key mechanisms:
  - dma_gather descgen runs on Q7 core pair (2q, 2q+1) selected by
    queue_num: queue_num=bank + num_swdge_queues=4 spreads SWDGE work over
    all four core pairs and gives each gather stream its own ring
    (eliminates ring-space stalls that dominated the 1-queue baseline).
  - Scatter matrices S are BINARY fp8e4m3 (exact 0/1) streamed from HBM;
    the dinv_dst scale is applied during the psum->sbuf copy against a
    broadcast dinvb tile (mixed fp16 lhsT x fp8 rhs matmul is exact).
  - Banks = src-node local quarters, so per-tile self-loop slots land in
    the same bank for every core; max-over-cores bucket padding drops from
    20% to ~6%.
  - Quarter AllGathers fire as soon as P1 writes each quarter of the
    node-major shard, overlapping the collective chain with the rest of
    P1 and the gather-phase start; gather calls for early banks are
    prefetched x3 so Pool descgen rarely waits on a collective.
  - P5 (residual update) is software-pipelined into the next layer's P1
    (P5 of chunk c+1 is emitted alongside conv/transpose of chunk c);
    the initial projection P0 is pipelined into layer 0's P1 the same way.
  - x0 lives in HBM (streamed per chunk in P5); conv_b/lin_b are dropped
    (absorbed exactly by training-mode BN over the node dim).
Per layer: P1 conv + PE-transpose + dinv_src scale -> 4 hsh quarters ->
quarter AGs -> P3 dma_gather + S-matmul aggregation into per-tile psum,
lin matmul per 2-tile window, h2 fp16 resident, BN stats via accum_out ->
P4 [128,2] stats AllReduce + BN fold -> P5 fused into next P1.
Classifier feature-major, f32 out [2, 12544]/core.
"""

import os
import sys

import numpy as np

for _p in ("/opt/trn_rl_repo", "/root/.axon_site/_ro/trn_rl_repo"):
    if os.path.isdir(_p) and _p not in sys.path:
        sys.path.append(_p)

import concourse.bass as bass
import concourse.bacc as bacc
import concourse.mybir as mybir
import concourse.tile as tile
from concourse import bass_utils

F32 = mybir.dt.float32
F16 = mybir.dt.float16
F8 = mybir.dt.float8e4
I16 = mybir.dt.int16
AF = mybir.ActivationFunctionType
OP = mybir.AluOpType
AX = mybir.AxisListType

N = 100000
NCORES = 8
NSH = N // NCORES
TILES = 98
NPAD = TILES * 128
NP = NCORES * NPAD
BANKS = 4
BLOC = [3200, 3200, 3200, 2944]     # local rows per bank (128-aligned)
BOFF = [0, 3200, 6400, 9600]
H = 128
L, HC, C = 4, 64, 2
ALPHA, THETA, EPS = 0.1, 0.5, 1e-5
C1 = float(1.0 - ALPHA - THETA)
WTILES = 2
GSUB = 16         # chunks per dma_gather call


# ----------------------------------------------------------------------------
# Host preprocessing (SPMD-common schedule)
# ----------------------------------------------------------------------------

def host_prep(edge_index):
    src0 = np.asarray(edge_index[0], np.int64)
    dst0 = np.asarray(edge_index[1], np.int64)
    loops = np.arange(N, dtype=np.int64)
    src = np.concatenate([src0, loops])
    dst = np.concatenate([dst0, loops])

    deg = np.bincount(dst0, minlength=N).astype(np.float32) + 1.0
    dinv = (1.0 / np.sqrt(deg)).astype(np.float32)

    core = dst // NSH
    tile_id = (dst - core * NSH) >> 7
    part = (dst - core * NSH) & 127
    j = src % NSH
    bank = np.minimum(j // 3200, 3)
    blocs = np.array(BLOC, np.int64)
    boffs = np.array(BOFF, np.int64)
    bidx = ((src // NSH) * blocs[bank] + (j - boffs[bank])).astype(np.int64)

    # counts per (core, bank, tile)
    cnt = np.zeros((NCORES, BANKS, TILES), np.int64)
    np.add.at(cnt, (core, bank, tile_id), 1)
    nmax = cnt.max(axis=0)                      # [BANKS, TILES]

    # common slot layout: bank stream = concat_t nmax[b, t]; bank end padded
    # to a multiple of 128.
    bucket0 = np.zeros((BANKS, TILES + 1), np.int64)   # slot offset in bank
    bank_slots = np.zeros(BANKS, np.int64)
    for b in range(BANKS):
        bucket0[b, 1:] = np.cumsum(nmax[b])
        bank_slots[b] = ((bucket0[b, -1] + 127) // 128) * 128
    bank_c = bank_slots // 128                  # chunks per bank
    bank_chunk0 = np.concatenate([[0], np.cumsum(bank_c)])
    Ctot = int(bank_c.sum())
    TOT = Ctot * 128
    bank_slot0 = bank_chunk0 * 128              # global slot offset per bank

    # common (chunk, tile) schedule, chunk-major si enumeration
    sched = []        # (global_chunk, tile, si, bank)
    si = 0
    tile_nmm = np.zeros(TILES, np.int64)
    for b in range(BANKS):
        for ch in range(int(bank_c[b])):
            s0, s1 = ch * 128, (ch + 1) * 128
            t0 = int(np.searchsorted(bucket0[b], s0, side="right") - 1)
            t1 = int(np.searchsorted(bucket0[b], s1 - 1, side="right") - 1)
            t1 = min(t1, TILES - 1)
            for t in range(t0, t1 + 1):
                if bucket0[b, t + 1] > s0 and bucket0[b, t] < s1 \
                        and nmax[b, t] > 0:
                    sched.append((int(bank_chunk0[b] + ch), t, si, b))
                    tile_nmm[t] += 1
                    si += 1
    NS = si

    # per-core slot fill
    order = np.argsort(((core * BANKS + bank) * TILES + tile_id), kind="stable")
    co, bo, to_, po, io = (core[order], bank[order], tile_id[order],
                           part[order], bidx[order])
    # position within (core, bank, tile)
    keyo = ((co * BANKS + bo) * TILES + to_)
    runstart = np.r_[0, np.flatnonzero(np.diff(keyo)) + 1]
    runid = np.zeros(len(keyo), np.int64)
    runid[runstart[1:]] = 1
    runid = np.cumsum(runid)
    pos = np.arange(len(keyo)) - runstart[runid]
    slot_global = bank_slot0[bo] + bucket0[bo, to_] + pos

    si_map = {}
    for (chg, t, s, b) in sched:
        si_map[(chg, t)] = s

    cores = []
    for c in range(NCORES):
        sel = co == c
        sl = slot_global[sel]
        idx = np.zeros(TOT, np.int16)
        idx[sl] = io[sel].astype(np.int16)
        Sarr = np.zeros((128, NS * 128), np.uint8)
        ch_of = sl >> 7
        si_of = np.array([si_map[(int(chv), int(tv))]
                          for chv, tv in zip(ch_of, to_[sel])], np.int64)
        Sarr[sl & 127, (si_of << 7) + po[sel]] = 0x38  # fp8e4m3 1.0

        idx16 = idx.reshape(TOT // 16, 16).T
        idx16 = np.tile(idx16, (8, 1)).astype(np.int16)
        cores.append(dict(idx16=np.ascontiguousarray(idx16),
                          S=np.ascontiguousarray(Sarr)))

    meta = dict(Ctot=Ctot, NS=NS, sched=sched, tile_nmm=tile_nmm,
                bank_c=bank_c.astype(np.int64),
                bank_chunk0=bank_chunk0.astype(np.int64))
    return dinv, meta, cores


def pack_weights(inputs):
    cols = [np.asarray(inputs["proj_W"], np.float32)]
    for l in range(L):
        cols.append(np.asarray(inputs["conv_W"][l], np.float32))
    for l in range(L):
        cols.append(np.asarray(inputs["lin_W"][l], np.float32))
    cols.append(np.asarray(inputs["cls_W1"], np.float32))
    w2 = np.zeros((H, C), np.float32)
    w2[:HC] = np.asarray(inputs["cls_W2"], np.float32)
    cols.append(w2)
    W = np.concatenate(cols, axis=1).astype(np.float16)
    nb = np.zeros((H, 7), np.float32)
    nb[:, 0] = np.asarray(inputs["proj_b"], np.float32)
    nb[:HC, 5] = np.asarray(inputs["cls_b1"], np.float32)
    nb[:C, 6] = np.asarray(inputs["cls_b2"], np.float32)
    bn = np.zeros((H, 2 * L), np.float32)
    for l in range(L):
        bn[:, l] = np.asarray(inputs["bn_g"][l], np.float32)
        bn[:, L + l] = np.asarray(inputs["bn_b"][l], np.float32)
    return W, nb, bn


# ----------------------------------------------------------------------------
# Device program
# ----------------------------------------------------------------------------

def build_program(meta):
    Ctot, NS = meta["Ctot"], meta["NS"]
    sched = meta["sched"]
    tile_nmm = meta["tile_nmm"]
    bank_c = meta["bank_c"]
    bank_chunk0 = meta["bank_chunk0"]
    WCOLS = H * (1 + 2 * L) + HC + C
    IDXCOLS = Ctot * 128 // 16

    nc = bacc.Bacc("TRN2", target_bir_lowering=False, debug=False,
                   enable_asserts=False, num_devices=NCORES,
                   num_swdge_queues=4)

    xT_in = nc.dram_tensor("xT_in", [H, NPAD], F16, kind="ExternalInput").ap()
    dinv_in = nc.dram_tensor("dinv_in", [H, TILES], F32,
                             kind="ExternalInput").ap()
    idx_in = nc.dram_tensor("idx_in", [H, IDXCOLS], I16,
                            kind="ExternalInput").ap()
    s_in = nc.dram_tensor("s_in", [H, NS * H], F8, kind="ExternalInput").ap()
    dinvb_in = nc.dram_tensor("dinvb_in", [H, NPAD], F16,
                              kind="ExternalInput").ap()
    w_in = nc.dram_tensor("w_in", [H, WCOLS], F16, kind="ExternalInput").ap()
    b_in = nc.dram_tensor("b_in", [H, 7], F32, kind="ExternalInput").ap()
    bn_in = nc.dram_tensor("bn_in", [H, 2 * L], F32, kind="ExternalInput").ap()
    out_d = nc.dram_tensor("out_d", [C, NPAD], F32, kind="ExternalOutput").ap()

    hsh_ds = [nc.dram_tensor(f"hsh{b}_d", [BLOC[b], H], F16,
                             kind="Internal").ap() for b in range(BANKS)]
    x0_d = nc.dram_tensor("x0_d", [H, NPAD], F16, kind="Internal").ap()
    hall_ds = [nc.dram_tensor(f"hall{b}_d", [NCORES * BLOC[b], H], F16,
                              kind="Internal", addr_space="Shared").ap()
               for b in range(BANKS)]
    stin_d = nc.dram_tensor("stin_d", [H, 2], F32, kind="Internal").ap()
    stout_d = nc.dram_tensor("stout_d", [H, 2], F32, kind="Internal",
                             addr_space="Shared").ap()

    xT = nc.alloc_sbuf_tensor("xT", [H, NPAD], F16).ap()
    dinvb = nc.alloc_sbuf_tensor("dinvb", [H, NPAD], F16).ap()
    h2 = nc.alloc_sbuf_tensor("h2", [H, NPAD], F16).ap()
    idxs = nc.alloc_sbuf_tensor("idxs", [H, IDXCOLS], I16).ap()
    wsb = nc.alloc_sbuf_tensor("wsb", [H, WCOLS], F16).ap()
    bsb = nc.alloc_sbuf_tensor("bsb", [H, 7], F32).ap()
    bnsb = nc.alloc_sbuf_tensor("bnsb", [H, 2 * L], F32).ap()
    dinv = nc.alloc_sbuf_tensor("dinv", [H, TILES], F32).ap()
    ident = nc.alloc_sbuf_tensor("ident", [H, H], F16).ap()
    sums = nc.alloc_sbuf_tensor("sums", [H, 64], F32).ap()
    sqs = nc.alloc_sbuf_tensor("sqs", [H, 64], F32).ap()
    stat = nc.alloc_sbuf_tensor("stat", [H, 12], F32).ap()

    wproj = wsb[:, 0:H]
    wconv = lambda l: wsb[:, H * (1 + l):H * (2 + l)]
    wlin = lambda l: wsb[:, H * (1 + L + l):H * (2 + L + l)]
    wcls1 = wsb[:, H * (1 + 2 * L):H * (1 + 2 * L) + HC]
    wcls2 = wsb[:HC, H * (1 + 2 * L) + HC:WCOLS]

    rg = [list(range(NCORES))]
    hall_banks = hall_ds
    PCH = [(o, min(512, NPAD - o)) for o in range(0, NPAD, 512)]
    AGCI = {6: 0, 12: 1, 18: 2, 24: 3}

    # window structure: WTILES tiles each; per (window, bank): matmul list
    windows = []
    for t0 in range(0, TILES, WTILES):
        t1 = min(t0 + WTILES, TILES)
        per_bank = []
        for b in range(BANKS):
            mms = [e for e in sched if e[3] == b and t0 <= e[1] < t1]
            per_bank.append(mms)
        windows.append((t0, t1, per_bank))

    with tile.TileContext(nc) as tc:
        # ================= P0 =================
        with tc.sbuf_pool(name="p0", bufs=3) as pool, \
             tc.psum_pool(name="p0p", bufs=2) as pp:
            nc.sync.dma_start(wsb, w_in)
            nc.sync.dma_start(bsb, b_in)
            nc.sync.dma_start(bnsb, bn_in)
            nc.sync.dma_start(dinv, dinv_in)
            nc.sync.dma_start(idxs, idx_in)
            nc.sync.dma_start(dinvb, dinvb_in)
            iota = pool.tile([H, H], F32, tag="iota")
            pidx = pool.tile([H, H], F32, tag="pidx")
            nc.gpsimd.iota(iota, pattern=[[1, H]], base=0, channel_multiplier=0,
                           allow_small_or_imprecise_dtypes=True)
            nc.gpsimd.iota(pidx, pattern=[[0, H]], base=0, channel_multiplier=1,
                           allow_small_or_imprecise_dtypes=True)
            nc.vector.tensor_tensor(ident, iota, pidx, OP.is_equal)

        def emit_p0_chunk(pool, pp, off, w):
            xin = pool.tile([H, 512], F16, tag="xin")
            nc.sync.dma_start(xin[:, :w], xT_in[:, off:off + w])
            ps = pp.tile([H, 512], F32, tag="ps0")
            nc.tensor.matmul(ps[:, :w], wproj, xin[:, :w])
            nc.scalar.activation(xT[:, off:off + w], ps[:, :w], AF.Relu,
                                 bias=bsb[:, 0:1], scale=1.0)
            x0t = pool.tile([H, 512], F16, tag="x0t")
            nc.scalar.mul(x0t[:, :w], xT[:, off:off + w], ALPHA)
            if off + w > NSH:
                nc.vector.memset(x0t[:, max(0, NSH - off):w], 0.0)
                nc.vector.memset(xT[:, NSH:off + w], 0.0)
            nc.sync.dma_start(x0_d[:, off:off + w], x0t[:, :w])

        def emit_p5_chunk(pool, off, w):
            x0t = pool.tile([H, 512], F16, tag="x0t5")
            nc.sync.dma_start(x0t[:, :w], x0_d[:, off:off + w])
            t1_ = pool.tile([H, 512], F32, tag="t1")
            nc.vector.tensor_scalar(t1_[:, :w], h2[:, off:off + w],
                                    stat[:, 6:7], stat[:, 7:8],
                                    op0=OP.mult, op1=OP.add)
            t2 = pool.tile([H, 512], F32, tag="t2")
            nc.vector.scalar_tensor_tensor(
                t2[:, :w], xT[:, off:off + w], THETA, t1_[:, :w],
                op0=OP.mult, op1=OP.add)
            t3 = pool.tile([H, 512], F32, tag="t3")
            nc.vector.tensor_tensor(t3[:, :w], t2[:, :w], x0t[:, :w], OP.add)
            nc.scalar.activation(xT[:, off:off + w], t3[:, :w], AF.Relu,
                                 bias=0.0, scale=1.0)
            if off + w > NSH:
                nc.vector.memset(xT[:, NSH:off + w], 0.0)

        for li in range(L):
            # ---- P1 (software-pipelined with P5 of previous layer) ----
            with tc.sbuf_pool(name=f"l{li}a", bufs=3) as pool, \
                 tc.psum_pool(name=f"l{li}ap", bufs=2) as pp, \
                 tc.psum_pool(name=f"l{li}at", bufs=4) as pt:
                if li > 0:
                    emit_p5_chunk(pool, *PCH[0])
                else:
                    emit_p0_chunk(pool, pp, *PCH[0])
                for ci, (off, w) in enumerate(PCH):
                    if ci + 1 < len(PCH):
                        if li > 0:
                            emit_p5_chunk(pool, *PCH[ci + 1])
                        else:
                            emit_p0_chunk(pool, pp, *PCH[ci + 1])
                    ps = pp.tile([H, 512], F32, tag="ps")
                    nc.tensor.matmul(ps[:, :w], wconv(li), xT[:, off:off + w])
                    hT = pool.tile([H, 512], F16, tag="hT")
                    nc.vector.tensor_copy(hT[:, :w], ps[:, :w])
                    stg = pool.tile([H, 512], F16, tag="stg")
                    for j in range(w // 128):
                        t = off // 128 + j
                        tp2 = pt.tile([H, H], F16, tag="tp2")
                        nc.tensor.transpose(tp2, hT[:, j * 128:(j + 1) * 128],
                                            ident)
                        nc.scalar.mul(stg[:, j * 128:(j + 1) * 128], tp2,
                                      dinv[:, t:t + 1])
                    # write rows [off, off+w) split by bank boundary
                    r = off
                    while r < off + w:
                        b = min(r // 3200, 3)
                        rend = min(off + w, BOFF[b] + BLOC[b])
                        dram = hsh_ds[b][r - BOFF[b]:rend - BOFF[b], :]\
                            .rearrange("(j p) f -> p j f", p=128)
                        c0, c1 = r - off, rend - off
                        nc.sync.dma_start(
                            dram, stg[:, c0:c1].rearrange(
                                "p (j f) -> p j f", f=H))
                        r = rend
                    if ci in AGCI:
                        k = AGCI[ci]
                        nc.gpsimd.collective_compute(
                            "AllGather", OP.bypass, replica_groups=rg,
                            ins=[hsh_ds[k]], outs=[hall_ds[k]])

            # ---- P3 ----
            with tc.sbuf_pool(name=f"l{li}g", bufs=3) as gpool, \
                 tc.sbuf_pool(name=f"l{li}s", bufs=3) as spool, \
                 tc.sbuf_pool(name=f"l{li}h", bufs=3) as hpool, \
                 tc.psum_pool(name=f"l{li}pa", bufs=5) as ppa, \
                 tc.psum_pool(name=f"l{li}pl", bufs=2) as ppl:
                msgs_of = {}       # global chunk -> (tile_handle, local_off)
                next_call = [int(bank_chunk0[b]) for b in range(BANKS)]

                def issue_call(b):
                    c0 = next_call[b]
                    cn = min(GSUB, int(bank_chunk0[b + 1]) - c0)
                    if cn <= 0:
                        return
                    mt = gpool.tile([H, GSUB * H], F16, tag=f"m{b}")
                    nc.gpsimd.dma_gather(
                        out_ap=mt[:, :cn * H].rearrange("p (c f) -> p c f",
                                                        f=H),
                        in_ap=hall_banks[b],
                        idxs_ap=idxs[:, c0 * 8:(c0 + cn) * 8],
                        num_idxs=cn * 128, num_idxs_reg=cn * 128,
                        elem_size=H, single_packet=False, queue_num=b)
                    for k in range(cn):
                        msgs_of[c0 + k] = (mt, k)
                    next_call[b] = c0 + cn

                def ensure_gathered(chg, b):
                    while chg >= next_call[b]:
                        issue_call(b)

                for _ in range(3):
                    for b in (0, 1, 2):
                        issue_call(b)

                tile_done = np.zeros(TILES, np.int64)
                psum_of = {}
                sc = 0
                for (t0, t1, per_bank) in windows:
                    for b in range(BANKS):
                        mms = per_bank[b]
                        if not mms:
                            continue
                        si0, si1 = mms[0][2], mms[-1][2] + 1
                        st = spool.tile([H, 16 * H], F8, tag=f"s{b}")
                        ns = si1 - si0
                        assert ns <= 16, (t0, b, ns)
                        nc.sync.dma_start(st[:, :ns * H],
                                          s_in[:, si0 * H:si1 * H])
                        for (chg, t, si, _b) in mms:
                            ensure_gathered(chg, b)
                            mt, lo = msgs_of[chg]
                            if t not in psum_of:
                                psum_of[t] = ppa.tile([H, H], F32, tag="agg",
                                                      name="agg")
                            first = tile_done[t] == 0
                            last = tile_done[t] == tile_nmm[t] - 1
                            nc.tensor.matmul(
                                psum_of[t],
                                mt[:, lo * H:(lo + 1) * H],
                                st[:, (si - si0) * H:(si - si0 + 1) * H],
                                start=bool(first), stop=bool(last),
                                skip_group_check=True)
                            tile_done[t] += 1
                    # finalize closed tiles of this window
                    nwc = (t1 - t0) * H
                    aggs = hpool.tile([H, WTILES * H], F16, tag="aggs")
                    for t in range(t0, t1):
                        if t in psum_of:
                            nc.vector.tensor_tensor(
                                aggs[:, (t - t0) * H:(t - t0 + 1) * H],
                                psum_of.pop(t),
                                dinvb[:, t * H:(t + 1) * H], OP.mult)
                        else:
                            nc.vector.memset(
                                aggs[:, (t - t0) * H:(t - t0 + 1) * H], 0.0)
                    ps3 = ppl.tile([H, WTILES * H], F32, tag="ps3")
                    nc.tensor.matmul(ps3[:, :nwc], wlin(li), aggs[:, :nwc])
                    gcol = t0 * H
                    nc.vector.tensor_scalar(
                        h2[:, gcol:gcol + nwc], ps3[:, :nwc], 0.0, None,
                        op0=OP.add, op1=OP.add,
                        accum_out=sums[:, sc:sc + 1])
                    sq = hpool.tile([H, WTILES * H], F16, tag="sq")
                    nc.vector.scalar_tensor_tensor(
                        sq[:, :nwc], h2[:, gcol:gcol + nwc], 0.0,
                        h2[:, gcol:gcol + nwc],
                        op0=OP.add, op1=OP.mult, accum_out=sqs[:, sc:sc + 1])
                    sc += 1
                    if sc > 64:
                        raise RuntimeError("stats overflow")
                nparts = sc

            # ---- P4 ----
            with tc.sbuf_pool(name=f"l{li}r", bufs=2) as pool:
                nc.vector.tensor_reduce(stat[:, 0:1], sums[:, :nparts], AX.X,
                                        OP.add)
                nc.vector.tensor_reduce(stat[:, 1:2], sqs[:, :nparts], AX.X,
                                        OP.add)
                nc.sync.dma_start(stin_d, stat[:, 0:2])
                nc.gpsimd.collective_compute(
                    "AllReduce", OP.add, replica_groups=rg,
                    ins=[stin_d], outs=[stout_d])
                nc.sync.dma_start(stat[:, 2:4], stout_d)
                invn = 1.0 / float(N)
                nc.vector.tensor_scalar_mul(stat[:, 4:5], stat[:, 2:3], invn)
                m2 = pool.tile([H, 1], F32)
                nc.vector.tensor_tensor(m2, stat[:, 4:5], stat[:, 4:5],
                                        OP.mult)
                nc.vector.scalar_tensor_tensor(stat[:, 5:6], stat[:, 3:4],
                                               invn, m2, op0=OP.mult,
                                               op1=OP.subtract)
                vps = pool.tile([H, 1], F32)
                nc.vector.tensor_scalar_add(vps, stat[:, 5:6], float(EPS))
                sd = pool.tile([H, 1], F32)
                nc.scalar.sqrt(sd, vps)
                inv = pool.tile([H, 1], F32)
                nc.vector.reciprocal(inv, sd)
                gi = pool.tile([H, 1], F32)
                nc.vector.tensor_tensor(gi, inv, bnsb[:, li:li + 1], OP.mult)
                nc.vector.tensor_scalar_mul(stat[:, 6:7], gi, C1)
                ms = pool.tile([H, 1], F32)
                nc.vector.tensor_tensor(ms, stat[:, 4:5], stat[:, 6:7],
                                        OP.mult)
                nc.vector.scalar_tensor_tensor(
                    stat[:, 7:8], bnsb[:, L + li:L + li + 1], C1, ms,
                    op0=OP.mult, op1=OP.subtract)

        # ================= P6 (fused with final P5) =================
        with tc.sbuf_pool(name="p6", bufs=3) as pool, \
             tc.psum_pool(name="p6p", bufs=2) as pp, \
             tc.psum_pool(name="p6q", bufs=2) as pq:
            emit_p5_chunk(pool, *PCH[0])
            for ci, (off, w) in enumerate(PCH):
                if ci + 1 < len(PCH):
                    emit_p5_chunk(pool, *PCH[ci + 1])
                ps = pp.tile([HC, 512], F32, tag="ps")
                nc.tensor.matmul(ps[:, :w], wcls1, xT[:, off:off + w])
                h3 = pool.tile([HC, 512], F16, tag="h3")
                nc.scalar.activation(h3[:, :w], ps[:, :w], AF.Relu,
                                     bias=bsb[:HC, 5:6], scale=1.0)
                ps2 = pq.tile([C, 512], F32, tag="ps2")
                nc.tensor.matmul(ps2[:, :w], wcls2, h3[:, :w])
                ot = pool.tile([C, 512], F32, tag="ot")
                nc.vector.tensor_scalar(ot[:, :w], ps2[:, :w],
                                        bsb[:C, 6:7], None, op0=OP.add)
                nc.sync.dma_start(out_d[:, off:off + w], ot[:, :w])

    nc.compile()
    return nc


# ----------------------------------------------------------------------------
# Orchestration
# ----------------------------------------------------------------------------

LAST_RESULTS = None
_PROGRAM_CACHE = {}


def kernel(**inputs):
    global LAST_RESULTS
    edge_index = np.asarray(inputs["edge_index"])
    dinv, meta, cores = host_prep(edge_index)
    W, NB, BN = pack_weights(inputs)

    x = np.asarray(inputs["x"], np.float32)
    dl = np.ones(NCORES * NPAD, np.float32)
    for c in range(NCORES):
        dl[c * NPAD:c * NPAD + NSH] = dinv[c * NSH:(c + 1) * NSH]
    dinv_nm = dl.reshape(NCORES, TILES, 128).transpose(0, 2, 1)
    dz = np.zeros(NCORES * NPAD, np.float16)
    for c in range(NCORES):
        dz[c * NPAD:c * NPAD + NSH] = dinv[c * NSH:(c + 1) * NSH]
    dinvb_nm = dz.reshape(NCORES, NPAD)

    in_maps = []
    for c in range(NCORES):
        xs = np.zeros((H, NPAD), np.float16)
        xs[:, :NSH] = x[c * NSH:(c + 1) * NSH].T.astype(np.float16)
        in_maps.append({
            "xT_in": xs,
            "dinv_in": np.ascontiguousarray(dinv_nm[c]),
            "idx_in": cores[c]["idx16"],
            "s_in": cores[c]["S"],
            "dinvb_in": np.ascontiguousarray(
                np.broadcast_to(dinvb_nm[c], (H, NPAD))),
            "w_in": W,
            "b_in": NB,
            "bn_in": BN,
        })

    key = (meta["Ctot"], meta["NS"])
    if key not in _PROGRAM_CACHE:
        _PROGRAM_CACHE[key] = build_program(meta)
    nc = _PROGRAM_CACHE[key]

    trace = bool(int(os.environ.get("GCN_TRACE", "0")))
    res = bass_utils.run_bass_kernel_spmd(
        nc, in_maps, core_ids=list(range(NCORES)), trace=trace)
    LAST_RESULTS = res

    out = np.empty((N, C), np.float32)
    for c in range(NCORES):
        o = res.results[c]["out_d"]
        out[c * NSH:(c + 1) * NSH] = o[:, :NSH].T
    return out


# revision 26
# speedup vs baseline: 1.1146x; 1.0796x over previous
"""Trainium2 Bass kernel v3 for DeepGCN (nn_DeepGCN_82454782148693).

8-core SPMD, dst-sharded, fp16 data path with binary-fp8 scatter matrices.
9.32ms (baseline) -> 3.17ms on trn2.8x1. Key mechanisms:
  - dma_gather descgen runs on Q7 core pair (2q, 2q+1) selected by
    queue_num: queue_num=bank + num_swdge_queues=4 spreads SWDGE work over
    all four core pairs and gives each gather stream its own ring
    (eliminates ring-space stalls that dominated the 1-queue baseline).
  - Scatter matrices S are BINARY fp8e4m3 (exact 0/1) streamed from HBM;
    the dinv_dst scale is applied during the psum->sbuf copy against a
    broadcast dinvb tile (mixed fp16 lhsT x fp8 rhs matmul is exact).
  - Banks = src-node local quarters, so per-tile self-loop slots land in
    the same bank for every core; max-over-cores bucket padding drops from
    20% to ~6%.
  - Quarter AllGathers fire as soon as P1 writes each quarter of the
    node-major shard, overlapping the collective chain with the rest of
    P1 and the gather-phase start; gather calls for early banks are
    prefetched x3 so Pool descgen rarely waits on a collective.
  - P5 (residual update) is software-pipelined into the next layer's P1
    (P5 of chunk c+1 is emitted alongside conv/transpose of chunk c);
    the initial projection P0 is pipelined into layer 0's P1 the same way.
  - x0 lives in HBM (streamed per chunk in P5); conv_b/lin_b are dropped
    (absorbed exactly by training-mode BN over the node dim).
Per layer: P1 conv + PE-transpose + dinv_src scale -> 4 hsh quarters ->
quarter AGs -> P3 dma_gather + S-matmul aggregation into per-tile psum,
lin matmul per 2-tile window, h2 fp16 resident, BN stats via accum_out ->
P4 [128,2] stats AllReduce + BN fold -> P5 fused into next P1.
Classifier feature-major, f32 out [2, 12544]/core.
"""

import os
import sys

import numpy as np

for _p in ("/opt/trn_rl_repo", "/root/.axon_site/_ro/trn_rl_repo"):
    if os.path.isdir(_p) and _p not in sys.path:
        sys.path.append(_p)

import concourse.bass as bass
import concourse.bacc as bacc
import concourse.mybir as mybir
import concourse.tile as tile
from concourse import bass_utils

F32 = mybir.dt.float32
F16 = mybir.dt.float16
F8 = mybir.dt.float8e4
I16 = mybir.dt.int16
AF = mybir.ActivationFunctionType
OP = mybir.AluOpType
AX = mybir.AxisListType

N = 100000
NCORES = 8
NSH = N // NCORES
TILES = 98
NPAD = TILES * 128
NP = NCORES * NPAD
BANKS = 4
BLOC = [3200, 3200, 3200, 2944]     # local rows per bank (128-aligned)
BOFF = [0, 3200, 6400, 9600]
H = 128
L, HC, C = 4, 64, 2
ALPHA, THETA, EPS = 0.1, 0.5, 1e-5
C1 = float(1.0 - ALPHA - THETA)
WTILES = 2
GSUB = 16         # chunks per dma_gather call


# ----------------------------------------------------------------------------
# Host preprocessing (SPMD-common schedule)
# ----------------------------------------------------------------------------

def host_prep(edge_index):
    src = np.asarray(edge_index[0], np.int64)
    dst = np.asarray(edge_index[1], np.int64)

    deg = np.bincount(dst, minlength=N).astype(np.float32) + 1.0
    dinv = (1.0 / np.sqrt(deg)).astype(np.float32)

    core = dst // NSH
    tile_id = (dst - core * NSH) >> 7
    part = (dst - core * NSH) & 127
    j = src % NSH
    bank = np.minimum(j // 3200, 3)
    blocs = np.array(BLOC, np.int64)
    boffs = np.array(BOFF, np.int64)
    bidx = ((src // NSH) * blocs[bank] + (j - boffs[bank])).astype(np.int64)

    # counts per (core, bank, tile)
    cnt = np.zeros((NCORES, BANKS, TILES), np.int64)
    np.add.at(cnt, (core, bank, tile_id), 1)
    nmax = cnt.max(axis=0)                      # [BANKS, TILES]

    # common slot layout: bank stream = concat_t nmax[b, t]; bank end padded
    # to a multiple of 128.
    bucket0 = np.zeros((BANKS, TILES + 1), np.int64)   # slot offset in bank
    bank_slots = np.zeros(BANKS, np.int64)
    for b in range(BANKS):
        bucket0[b, 1:] = np.cumsum(nmax[b])
        bank_slots[b] = ((bucket0[b, -1] + 127) // 128) * 128
    bank_c = bank_slots // 128                  # chunks per bank
    bank_chunk0 = np.concatenate([[0], np.cumsum(bank_c)])
    Ctot = int(bank_c.sum())
    TOT = Ctot * 128
    bank_slot0 = bank_chunk0 * 128              # global slot offset per bank

    # common (chunk, tile) schedule, chunk-major si enumeration
    sched = []        # (global_chunk, tile, si, bank)
    si = 0
    tile_nmm = np.zeros(TILES, np.int64)
    for b in range(BANKS):
        for ch in range(int(bank_c[b])):
            s0, s1 = ch * 128, (ch + 1) * 128
            t0 = int(np.searchsorted(bucket0[b], s0, side="right") - 1)
            t1 = int(np.searchsorted(bucket0[b], s1 - 1, side="right") - 1)
            t1 = min(t1, TILES - 1)
            for t in range(t0, t1 + 1):
                if bucket0[b, t + 1] > s0 and bucket0[b, t] < s1 \
                        and nmax[b, t] > 0:
                    sched.append((int(bank_chunk0[b] + ch), t, si, b))
                    tile_nmm[t] += 1
                    si += 1
    NS = si

    # per-core slot fill
    order = np.argsort(((core * BANKS + bank) * TILES + tile_id), kind="stable")
    co, bo, to_, po, io = (core[order], bank[order], tile_id[order],
                           part[order], bidx[order])
    # position within (core, bank, tile)
    keyo = ((co * BANKS + bo) * TILES + to_)
    runstart = np.r_[0, np.flatnonzero(np.diff(keyo)) + 1]
    runid = np.zeros(len(keyo), np.int64)
    runid[runstart[1:]] = 1
    runid = np.cumsum(runid)
    pos = np.arange(len(keyo)) - runstart[runid]
    slot_global = bank_slot0[bo] + bucket0[bo, to_] + pos

    si_map = {}
    for (chg, t, s, b) in sched:
        si_map[(chg, t)] = s

    cores = []
    for c in range(NCORES):
        sel = co == c
        sl = slot_global[sel]
        idx = np.zeros(TOT, np.int16)
        idx[sl] = io[sel].astype(np.int16)
        Sarr = np.zeros((128, NS * 128), np.uint8)
        ch_of = sl >> 7
        si_of = np.array([si_map[(int(chv), int(tv))]
                          for chv, tv in zip(ch_of, to_[sel])], np.int64)
        Sarr[sl & 127, (si_of << 7) + po[sel]] = 0x38  # fp8e4m3 1.0

        idx16 = idx.reshape(TOT // 16, 16).T
        idx16 = np.tile(idx16, (8, 1)).astype(np.int16)
        cores.append(dict(idx16=np.ascontiguousarray(idx16),
                          S=np.ascontiguousarray(Sarr)))

    meta = dict(Ctot=Ctot, NS=NS, sched=sched, tile_nmm=tile_nmm,
                bank_c=bank_c.astype(np.int64),
                bank_chunk0=bank_chunk0.astype(np.int64))
    return dinv, meta, cores


def pack_weights(inputs):
    cols = [np.asarray(inputs["proj_W"], np.float32)]
    for l in range(L):
        cols.append(np.asarray(inputs["conv_W"][l], np.float32))
    for l in range(L):
        cols.append(np.asarray(inputs["lin_W"][l], np.float32))
    cols.append(np.asarray(inputs["cls_W1"], np.float32))
    w2 = np.zeros((H, C), np.float32)
    w2[:HC] = np.asarray(inputs["cls_W2"], np.float32)
    cols.append(w2)
    W = np.concatenate(cols, axis=1).astype(np.float16)
    nb = np.zeros((H, 7), np.float32)
    nb[:, 0] = np.asarray(inputs["proj_b"], np.float32)
    nb[:HC, 5] = np.asarray(inputs["cls_b1"], np.float32)
    nb[:C, 6] = np.asarray(inputs["cls_b2"], np.float32)
    bn = np.zeros((H, 2 * L), np.float32)
    for l in range(L):
        bn[:, l] = np.asarray(inputs["bn_g"][l], np.float32)
        bn[:, L + l] = np.asarray(inputs["bn_b"][l], np.float32)
    return W, nb, bn


# ----------------------------------------------------------------------------
# Device program
# ----------------------------------------------------------------------------

def build_program(meta):
    Ctot, NS = meta["Ctot"], meta["NS"]
    sched = meta["sched"]
    tile_nmm = meta["tile_nmm"]
    bank_c = meta["bank_c"]
    bank_chunk0 = meta["bank_chunk0"]
    WCOLS = H * (1 + 2 * L) + HC + C
    IDXCOLS = Ctot * 128 // 16

    nc = bacc.Bacc("TRN2", target_bir_lowering=False, debug=False,
                   enable_asserts=False, num_devices=NCORES,
                   num_swdge_queues=4)

    xT_in = nc.dram_tensor("xT_in", [H, NPAD], F16, kind="ExternalInput").ap()
    dinv_in = nc.dram_tensor("dinv_in", [H, TILES], F32,
                             kind="ExternalInput").ap()
    idx_in = nc.dram_tensor("idx_in", [H, IDXCOLS], I16,
                            kind="ExternalInput").ap()
    s_in = nc.dram_tensor("s_in", [H, NS * H], F8, kind="ExternalInput").ap()
    dinvb_in = nc.dram_tensor("dinvb_in", [H, NPAD], F16,
                              kind="ExternalInput").ap()
    w_in = nc.dram_tensor("w_in", [H, WCOLS], F16, kind="ExternalInput").ap()
    b_in = nc.dram_tensor("b_in", [H, 7], F32, kind="ExternalInput").ap()
    bn_in = nc.dram_tensor("bn_in", [H, 2 * L], F32, kind="ExternalInput").ap()
    out_d = nc.dram_tensor("out_d", [C, NPAD], F32, kind="ExternalOutput").ap()

    hsh_ds = [nc.dram_tensor(f"hsh{b}_d", [BLOC[b], H], F16,
                             kind="Internal").ap() for b in range(BANKS)]
    x0_d = nc.dram_tensor("x0_d", [H, NPAD], F16, kind="Internal").ap()
    hall_ds = [nc.dram_tensor(f"hall{b}_d", [NCORES * BLOC[b], H], F16,
                              kind="Internal", addr_space="Shared").ap()
               for b in range(BANKS)]
    stin_d = nc.dram_tensor("stin_d", [H, 2], F32, kind="Internal").ap()
    stout_d = nc.dram_tensor("stout_d", [H, 2], F32, kind="Internal",
                             addr_space="Shared").ap()

    xT = nc.alloc_sbuf_tensor("xT", [H, NPAD], F16).ap()
    dinvb = nc.alloc_sbuf_tensor("dinvb", [H, NPAD], F16).ap()
    h2 = nc.alloc_sbuf_tensor("h2", [H, NPAD], F16).ap()
    idxs = nc.alloc_sbuf_tensor("idxs", [H, IDXCOLS], I16).ap()
    wsb = nc.alloc_sbuf_tensor("wsb", [H, WCOLS], F16).ap()
    bsb = nc.alloc_sbuf_tensor("bsb", [H, 7], F32).ap()
    bnsb = nc.alloc_sbuf_tensor("bnsb", [H, 2 * L], F32).ap()
    dinv = nc.alloc_sbuf_tensor("dinv", [H, TILES], F32).ap()
    ident = nc.alloc_sbuf_tensor("ident", [H, H], F16).ap()
    sums = nc.alloc_sbuf_tensor("sums", [H, 64], F32).ap()
    sqs = nc.alloc_sbuf_tensor("sqs", [H, 64], F32).ap()
    stat = nc.alloc_sbuf_tensor("stat", [H, 12], F32).ap()

    wproj = wsb[:, 0:H]
    wconv = lambda l: wsb[:, H * (1 + l):H * (2 + l)]
    wlin = lambda l: wsb[:, H * (1 + L + l):H * (2 + L + l)]
    wcls1 = wsb[:, H * (1 + 2 * L):H * (1 + 2 * L) + HC]
    wcls2 = wsb[:HC, H * (1 + 2 * L) + HC:WCOLS]

    rg = [list(range(NCORES))]
    hall_banks = hall_ds
    PCH = [(o, min(512, NPAD - o)) for o in range(0, NPAD, 512)]
    AGCI = {6: 0, 12: 1, 18: 2, 24: 3}

    # window structure: WTILES tiles each; per (window, bank): matmul list
    windows = []
    for t0 in range(0, TILES, WTILES):
        t1 = min(t0 + WTILES, TILES)
        per_bank = []
        for b in range(BANKS):
            mms = [e for e in sched if e[3] == b and t0 <= e[1] < t1]
            per_bank.append(mms)
        windows.append((t0, t1, per_bank))

    with tile.TileContext(nc) as tc:
        # ================= P0 =================
        with tc.sbuf_pool(name="p0", bufs=3) as pool, \
             tc.psum_pool(name="p0p", bufs=2) as pp:
            nc.sync.dma_start(wsb, w_in)
            nc.sync.dma_start(bsb, b_in)
            nc.sync.dma_start(bnsb, bn_in)
            nc.sync.dma_start(dinv, dinv_in)
            nc.sync.dma_start(idxs, idx_in)
            nc.sync.dma_start(dinvb, dinvb_in)
            iota = pool.tile([H, H], F32, tag="iota")
            pidx = pool.tile([H, H], F32, tag="pidx")
            nc.gpsimd.iota(iota, pattern=[[1, H]], base=0, channel_multiplier=0,
                           allow_small_or_imprecise_dtypes=True)
            nc.gpsimd.iota(pidx, pattern=[[0, H]], base=0, channel_multiplier=1,
                           allow_small_or_imprecise_dtypes=True)
            nc.vector.tensor_tensor(ident, iota, pidx, OP.is_equal)

        def emit_p0_chunk(pool, pp, off, w):
            xin = pool.tile([H, 512], F16, tag="xin")
            nc.sync.dma_start(xin[:, :w], xT_in[:, off:off + w])
            ps = pp.tile([H, 512], F32, tag="ps0")
            nc.tensor.matmul(ps[:, :w], wproj, xin[:, :w])
            nc.scalar.activation(xT[:, off:off + w], ps[:, :w], AF.Relu,
                                 bias=bsb[:, 0:1], scale=1.0)
            x0t = pool.tile([H, 512], F16, tag="x0t")
            nc.scalar.mul(x0t[:, :w], xT[:, off:off + w], ALPHA)
            if off + w > NSH:
                nc.vector.memset(x0t[:, max(0, NSH - off):w], 0.0)
                nc.vector.memset(xT[:, NSH:off + w], 0.0)
            nc.sync.dma_start(x0_d[:, off:off + w], x0t[:, :w])

        def emit_p5_chunk(pool, off, w):
            x0t = pool.tile([H, 512], F16, tag="x0t5")
            nc.sync.dma_start(x0t[:, :w], x0_d[:, off:off + w])
            t1_ = pool.tile([H, 512], F32, tag="t1")
            nc.vector.tensor_scalar(t1_[:, :w], h2[:, off:off + w],
                                    stat[:, 6:7], stat[:, 7:8],
                                    op0=OP.mult, op1=OP.add)
            t2 = pool.tile([H, 512], F32, tag="t2")
            nc.vector.scalar_tensor_tensor(
                t2[:, :w], xT[:, off:off + w], THETA, t1_[:, :w],
                op0=OP.mult, op1=OP.add)
            t3 = pool.tile([H, 512], F32, tag="t3")
            nc.vector.tensor_tensor(t3[:, :w], t2[:, :w], x0t[:, :w], OP.add)
            nc.scalar.activation(xT[:, off:off + w], t3[:, :w], AF.Relu,
                                 bias=0.0, scale=1.0)
            if off + w > NSH:
                nc.vector.memset(xT[:, NSH:off + w], 0.0)

        for li in range(L):
            # ---- P1 (software-pipelined with P5 of previous layer) ----
            with tc.sbuf_pool(name=f"l{li}a", bufs=3) as pool, \
                 tc.psum_pool(name=f"l{li}ap", bufs=2) as pp, \
                 tc.psum_pool(name=f"l{li}at", bufs=4) as pt:
                if li > 0:
                    emit_p5_chunk(pool, *PCH[0])
                else:
                    emit_p0_chunk(pool, pp, *PCH[0])
                for ci, (off, w) in enumerate(PCH):
                    if ci + 1 < len(PCH):
                        if li > 0:
                            emit_p5_chunk(pool, *PCH[ci + 1])
                        else:
                            emit_p0_chunk(pool, pp, *PCH[ci + 1])
                    ps = pp.tile([H, 512], F32, tag="ps")
                    nc.tensor.matmul(ps[:, :w], wconv(li), xT[:, off:off + w])
                    hT = pool.tile([H, 512], F16, tag="hT")
                    nc.vector.tensor_copy(hT[:, :w], ps[:, :w])
                    nc.vector.tensor_tensor(h2[:, off:off + w], hT[:, :w],
                                            dinvb[:, off:off + w], OP.mult)
                    stg = pool.tile([H, 512], F16, tag="stg")
                    for j in range(w // 128):
                        t = off // 128 + j
                        tp2 = pt.tile([H, H], F16, tag="tp2")
                        nc.tensor.transpose(tp2, hT[:, j * 128:(j + 1) * 128],
                                            ident)
                        nc.scalar.mul(stg[:, j * 128:(j + 1) * 128], tp2,
                                      dinv[:, t:t + 1])
                    # write rows [off, off+w) split by bank boundary
                    r = off
                    while r < off + w:
                        b = min(r // 3200, 3)
                        rend = min(off + w, BOFF[b] + BLOC[b])
                        dram = hsh_ds[b][r - BOFF[b]:rend - BOFF[b], :]\
                            .rearrange("(j p) f -> p j f", p=128)
                        c0, c1 = r - off, rend - off
                        nc.sync.dma_start(
                            dram, stg[:, c0:c1].rearrange(
                                "p (j f) -> p j f", f=H))
                        r = rend
                    if ci in AGCI:
                        k = AGCI[ci]
                        nc.gpsimd.collective_compute(
                            "AllGather", OP.bypass, replica_groups=rg,
                            ins=[hsh_ds[k]], outs=[hall_ds[k]])

            # ---- P3 ----
            with tc.sbuf_pool(name=f"l{li}g", bufs=3) as gpool, \
                 tc.sbuf_pool(name=f"l{li}s", bufs=3) as spool, \
                 tc.sbuf_pool(name=f"l{li}h", bufs=3) as hpool, \
                 tc.psum_pool(name=f"l{li}pa", bufs=5) as ppa, \
                 tc.psum_pool(name=f"l{li}pl", bufs=2) as ppl:
                msgs_of = {}       # global chunk -> (tile_handle, local_off)
                next_call = [int(bank_chunk0[b]) for b in range(BANKS)]

                def issue_call(b):
                    c0 = next_call[b]
                    cn = min(GSUB, int(bank_chunk0[b + 1]) - c0)
                    if cn <= 0:
                        return
                    mt = gpool.tile([H, GSUB * H], F16, tag=f"m{b}")
                    nc.gpsimd.dma_gather(
                        out_ap=mt[:, :cn * H].rearrange("p (c f) -> p c f",
                                                        f=H),
                        in_ap=hall_banks[b],
                        idxs_ap=idxs[:, c0 * 8:(c0 + cn) * 8],
                        num_idxs=cn * 128, num_idxs_reg=cn * 128,
                        elem_size=H, single_packet=False, queue_num=b)
                    for k in range(cn):
                        msgs_of[c0 + k] = (mt, k)
                    next_call[b] = c0 + cn

                def ensure_gathered(chg, b):
                    while chg >= next_call[b]:
                        issue_call(b)

                for _ in range(3):
                    for b in (0, 1, 2):
                        issue_call(b)

                tile_done = np.zeros(TILES, np.int64)
                psum_of = {}
                sc = 0
                for (t0, t1, per_bank) in windows:
                    for b in range(BANKS):
                        mms = per_bank[b]
                        if not mms:
                            continue
                        si0, si1 = mms[0][2], mms[-1][2] + 1
                        st = spool.tile([H, 16 * H], F8, tag=f"s{b}")
                        ns = si1 - si0
                        assert ns <= 16, (t0, b, ns)
                        nc.sync.dma_start(st[:, :ns * H],
                                          s_in[:, si0 * H:si1 * H])
                        for (chg, t, si, _b) in mms:
                            ensure_gathered(chg, b)
                            mt, lo = msgs_of[chg]
                            if t not in psum_of:
                                psum_of[t] = ppa.tile([H, H], F32, tag="agg",
                                                      name="agg")
                            first = tile_done[t] == 0
                            last = tile_done[t] == tile_nmm[t] - 1
                            nc.tensor.matmul(
                                psum_of[t],
                                mt[:, lo * H:(lo + 1) * H],
                                st[:, (si - si0) * H:(si - si0 + 1) * H],
                                start=bool(first), stop=bool(last),
                                skip_group_check=True)
                            tile_done[t] += 1
                    # finalize closed tiles of this window
                    nwc = (t1 - t0) * H
                    aggs = hpool.tile([H, WTILES * H], F16, tag="aggs")
                    for t in range(t0, t1):
                        if t in psum_of:
                            t4 = hpool.tile([H, H], F32, tag="t4")
                            nc.vector.tensor_tensor(
                                t4, psum_of.pop(t),
                                h2[:, t * H:(t + 1) * H], OP.add)
                            nc.vector.tensor_tensor(
                                aggs[:, (t - t0) * H:(t - t0 + 1) * H],
                                t4, dinvb[:, t * H:(t + 1) * H], OP.mult)
                        else:
                            nc.vector.tensor_tensor(
                                aggs[:, (t - t0) * H:(t - t0 + 1) * H],
                                h2[:, t * H:(t + 1) * H],
                                dinvb[:, t * H:(t + 1) * H], OP.mult)
                    ps3 = ppl.tile([H, WTILES * H], F32, tag="ps3")
                    nc.tensor.matmul(ps3[:, :nwc], wlin(li), aggs[:, :nwc])
                    gcol = t0 * H
                    nc.vector.tensor_scalar(
                        h2[:, gcol:gcol + nwc], ps3[:, :nwc], 0.0, None,
                        op0=OP.add, op1=OP.add,
                        accum_out=sums[:, sc:sc + 1])
                    sq = hpool.tile([H, WTILES * H], F16, tag="sq")
                    nc.vector.scalar_tensor_tensor(
                        sq[:, :nwc], h2[:, gcol:gcol + nwc], 0.0,
                        h2[:, gcol:gcol + nwc],
                        op0=OP.add, op1=OP.mult, accum_out=sqs[:, sc:sc + 1])
                    sc += 1
                    if sc > 64:
                        raise RuntimeError("stats overflow")
                nparts = sc

            # ---- P4 ----
            with tc.sbuf_pool(name=f"l{li}r", bufs=2) as pool:
                nc.vector.tensor_reduce(stat[:, 0:1], sums[:, :nparts], AX.X,
                                        OP.add)
                nc.vector.tensor_reduce(stat[:, 1:2], sqs[:, :nparts], AX.X,
                                        OP.add)
                nc.sync.dma_start(stin_d, stat[:, 0:2])
                nc.gpsimd.collective_compute(
                    "AllReduce", OP.add, replica_groups=rg,
                    ins=[stin_d], outs=[stout_d])
                nc.sync.dma_start(stat[:, 2:4], stout_d)
                invn = 1.0 / float(N)
                nc.vector.tensor_scalar_mul(stat[:, 4:5], stat[:, 2:3], invn)
                m2 = pool.tile([H, 1], F32)
                nc.vector.tensor_tensor(m2, stat[:, 4:5], stat[:, 4:5],
                                        OP.mult)
                nc.vector.scalar_tensor_tensor(stat[:, 5:6], stat[:, 3:4],
                                               invn, m2, op0=OP.mult,
                                               op1=OP.subtract)
                vps = pool.tile([H, 1], F32)
                nc.vector.tensor_scalar_add(vps, stat[:, 5:6], float(EPS))
                sd = pool.tile([H, 1], F32)
                nc.scalar.sqrt(sd, vps)
                inv = pool.tile([H, 1], F32)
                nc.vector.reciprocal(inv, sd)
                gi = pool.tile([H, 1], F32)
                nc.vector.tensor_tensor(gi, inv, bnsb[:, li:li + 1], OP.mult)
                nc.vector.tensor_scalar_mul(stat[:, 6:7], gi, C1)
                ms = pool.tile([H, 1], F32)
                nc.vector.tensor_tensor(ms, stat[:, 4:5], stat[:, 6:7],
                                        OP.mult)
                nc.vector.scalar_tensor_tensor(
                    stat[:, 7:8], bnsb[:, L + li:L + li + 1], C1, ms,
                    op0=OP.mult, op1=OP.subtract)

        # ================= P6 (fused with final P5) =================
        with tc.sbuf_pool(name="p6", bufs=3) as pool, \
             tc.psum_pool(name="p6p", bufs=2) as pp, \
             tc.psum_pool(name="p6q", bufs=2) as pq:
            emit_p5_chunk(pool, *PCH[0])
            for ci, (off, w) in enumerate(PCH):
                if ci + 1 < len(PCH):
                    emit_p5_chunk(pool, *PCH[ci + 1])
                ps = pp.tile([HC, 512], F32, tag="ps")
                nc.tensor.matmul(ps[:, :w], wcls1, xT[:, off:off + w])
                h3 = pool.tile([HC, 512], F16, tag="h3")
                nc.scalar.activation(h3[:, :w], ps[:, :w], AF.Relu,
                                     bias=bsb[:HC, 5:6], scale=1.0)
                ps2 = pq.tile([C, 512], F32, tag="ps2")
                nc.tensor.matmul(ps2[:, :w], wcls2, h3[:, :w])
                ot = pool.tile([C, 512], F32, tag="ot")
                nc.vector.tensor_scalar(ot[:, :w], ps2[:, :w],
                                        bsb[:C, 6:7], None, op0=OP.add)
                nc.sync.dma_start(out_d[:, off:off + w], ot[:, :w])

    nc.compile()
    return nc


# ----------------------------------------------------------------------------
# Orchestration
# ----------------------------------------------------------------------------

LAST_RESULTS = None
_PROGRAM_CACHE = {}


def kernel(**inputs):
    global LAST_RESULTS
    edge_index = np.asarray(inputs["edge_index"])
    dinv, meta, cores = host_prep(edge_index)
    W, NB, BN = pack_weights(inputs)

    x = np.asarray(inputs["x"], np.float32)
    dl = np.ones(NCORES * NPAD, np.float32)
    for c in range(NCORES):
        dl[c * NPAD:c * NPAD + NSH] = dinv[c * NSH:(c + 1) * NSH]
    dinv_nm = dl.reshape(NCORES, TILES, 128).transpose(0, 2, 1)
    dz = np.zeros(NCORES * NPAD, np.float16)
    for c in range(NCORES):
        dz[c * NPAD:c * NPAD + NSH] = dinv[c * NSH:(c + 1) * NSH]
    dinvb_nm = dz.reshape(NCORES, NPAD)

    in_maps = []
    for c in range(NCORES):
        xs = np.zeros((H, NPAD), np.float16)
        xs[:, :NSH] = x[c * NSH:(c + 1) * NSH].T.astype(np.float16)
        in_maps.append({
            "xT_in": xs,
            "dinv_in": np.ascontiguousarray(dinv_nm[c]),
            "idx_in": cores[c]["idx16"],
            "s_in": cores[c]["S"],
            "dinvb_in": np.ascontiguousarray(
                np.broadcast_to(dinvb_nm[c], (H, NPAD))),
            "w_in": W,
            "b_in": NB,
            "bn_in": BN,
        })

    key = (meta["Ctot"], meta["NS"])
    if key not in _PROGRAM_CACHE:
        _PROGRAM_CACHE[key] = build_program(meta)
    nc = _PROGRAM_CACHE[key]

    trace = bool(int(os.environ.get("GCN_TRACE", "0")))
    res = bass_utils.run_bass_kernel_spmd(
        nc, in_maps, core_ids=list(range(NCORES)), trace=trace)
    LAST_RESULTS = res

    out = np.empty((N, C), np.float32)
    for c in range(NCORES):
        o = res.results[c]["out_d"]
        out[c * NSH:(c + 1) * NSH] = o[:, :NSH].T
    return out
